# revision 2
# baseline (speedup 1.0000x reference)
"""Trainium2 Bass kernel for GQA attention (nn_Attention_34832184770944).

Sharding: tensor-parallel across heads on 8 cores. Core m computes KV head m
and Q heads 4m..4m+3 for the full sequence, then one AllToAll exchanges
attention outputs so core m ends with ALL 32 heads for ITS 256-token slice
of each batch; the output projection is token-sharded with the full wo
(K=128 full-contraction GEMM, wo streamed by column chunks). Host
concatenates the 8 token shards.

Device layout:
  - QKV projection is e-major: lhsT = wqkv d-tile [128d,128e] (resident
    weights), rhs = xT chunk [128d,512t] -> psum [128e,512t]. q,k come out
    already transposed (e-major); only v needs a TensorE transpose.
  - RoPE: host permutes wq/wk columns per head to de-interleave (r parts in
    rows 0..31, i parts in rows 32..63), so RoPE is 6 elementwise ops per
    (head, 512-token chunk) split across DVE and GpSimd. Permutation
    cancels in q.k.
  - All per-chunk activations live in per-chunk tiles so attention on
    chunk j starts as soon as chunk j is projected+roped (no coarse deps),
    and attention is software-pipelined by one head so ScalarE's exp of
    head h overlaps PE's PV of head h-1.
  - scores^T tile: lhsT = rkT [64,128], rhs = rqT_hj [64,512-vs] (trimmed
    to the causally-valid tq range); exp on ScalarE (scale=1/8, no max
    subtraction); causal zeroing of the diagonal subtile on GpSimd
    (affine_select on the bf16 probabilities in SBUF).
  - PV: lhsT = V_aug [128tk,65] (col 64 = ones), rhs = P^T -> O^T [65,512];
    row 64 = softmax denominators; reciprocal (bf16) + rank-1 replicate
    matmul + DVE multiply -> oT_hj [64,512] bf16.
  - One AllToAll for both batches at end of compute: ccin [2048,512] =
    8 token-slices x (4 heads x 64) x (b0 cols | b1 cols) -> ccout
    [2048,512] = full 2048 e-rows for my two 256-token slices. A single
    collective: they share the SDMA path with kernel DMA, so overlapping
    them with compute slows both (measured 70us vs 21us per 1MB).
  - out-proj: psum [128tok,512d] accumulated over 16 e-tiles: lhsT =
    ccout tile [128e,128tok], rhs = wo chunk [128e,512].
"""

import os
import sys

sys.path.insert(0, "/opt/trn_rl_repo")

import numpy as np
import ml_dtypes

BF16 = ml_dtypes.bfloat16

B, S, D = 2, 2048, 2048
NH = 4              # q heads per core
HD = 64             # head dim
KD = D // 128       # 16 contraction tiles
NCH = S // 512      # 4 chunks of 512 tokens per batch
SCALE = 1.0 / 8.0
TOK = 2 * 256       # output tokens per core (256 per batch)


def _build_bass():
    import concourse.bacc as bacc
    import concourse.mybir as mybir
    from concourse.tile import TileContext
    from concourse.masks import make_identity

    f32 = mybir.dt.float32
    bf16 = mybir.dt.bfloat16
    Exp = mybir.ActivationFunctionType.Exp

    nc = bacc.Bacc(None, target_bir_lowering=False)
    xT_d = nc.dram_tensor("xT", [B, D, S], bf16, kind="ExternalInput")
    wqkv_d = nc.dram_tensor("wqkv", [D, 384], bf16, kind="ExternalInput")
    wo_d = nc.dram_tensor("wo_f", [D, D], bf16, kind="ExternalInput")
    # cos/sin tables duplicated into both 32-row halves so every RoPE
    # tensor_tensor op has partition-aligned SBUF inputs
    ce_d = nc.dram_tensor("ce", [64, S], bf16, kind="ExternalInput")
    se_d = nc.dram_tensor("se", [64, S], bf16, kind="ExternalInput")
    out_d = nc.dram_tensor("out", [TOK, D], f32, kind="ExternalOutput")

    with TileContext(nc) as tc:
        with (
            tc.tile_pool(name="const", bufs=1) as constp,
            tc.tile_pool(name="wts", bufs=1) as wtsp,
            tc.tile_pool(name="wo", bufs=32) as wop,
            tc.tile_pool(name="xin", bufs=20) as xinp,
            tc.tile_pool(name="act", bufs=1) as actp,
            tc.tile_pool(name="rope", bufs=8) as ropep,
            tc.tile_pool(name="pt", bufs=34) as ptp,
            tc.tile_pool(name="sml", bufs=3) as smlp,
            tc.tile_pool(name="oa", bufs=16) as oap,
            tc.tile_pool(name="ost", bufs=4) as ostp,
            tc.tile_pool(name="ps", bufs=7, space="PSUM") as psp,
            tc.tile_pool(name="dram", bufs=2, space="DRAM") as dramp,
        ):
            # ---- constants ----
            ident = constp.tile([64, 64], bf16, name="ident")
            make_identity(nc, ident[:, :])
            ones = constp.tile([1, 64], bf16, name="ones")
            nc.vector.memset(ones[:, :], 1.0)

            # ---- weights / freqs (persistent) ----
            wqkv_sb = wtsp.tile([128, KD * 384], bf16, name="wqkv_sb")
            for kd in range(KD):
                nc.sync.dma_start(
                    out=wqkv_sb[:, kd * 384 : kd * 384 + 384],
                    in_=wqkv_d[kd * 128 : kd * 128 + 128, :],
                )
            ce_sb = wtsp.tile([64, S], bf16, name="ce_sb")
            nc.sync.dma_start(out=ce_sb[:, :], in_=ce_d[:, :])
            se_sb = wtsp.tile([64, S], bf16, name="se_sb")
            nc.sync.dma_start(out=se_sb[:, :], in_=se_d[:, :])

            # one combined A2A for both batches: cols 0:256 = my b0 tokens,
            # cols 256:512 = my b1 tokens. A single end-of-compute collective:
            # collectives share the SDMA path with kernel DMA, so overlapping
            # them with compute slows both (measured 70us vs 21us per 1MB).
            ccin = dramp.tile([D, 512], bf16, tag="ccin", bufs=1, name="ccin")
            ccout = dramp.tile([D, 512], bf16, tag="ccout", bufs=1, name="ccout")
            for b in range(B):
                gps = nc.gpsimd
                rkTs, vsbs = [], []
                for c in range(NCH):
                    cs = slice(c * 512, c * 512 + 512)
                    # ---- projection for chunk c ----
                    xcs = []
                    for kd in range(KD):
                        xc = xinp.tile([128, 512], bf16, tag="xc", name="xc")
                        nc.sync.dma_start(
                            out=xc[:, :],
                            in_=xT_d[
                                b, kd * 128 : kd * 128 + 128,
                                c * 512 : c * 512 + 512,
                            ],
                        )
                        xcs.append(xc)
                    qcs, kc, vc = [], None, None
                    for t in range(3):
                        ps_p = psp.tile([128, 512], f32, tag="ps", bufs=2,
                                        name="ps_p")
                        for kd in range(KD):
                            nc.tensor.matmul(
                                ps_p[:, :],
                                lhsT=wqkv_sb[
                                    :, kd * 384 + t * 128 : kd * 384 + t * 128 + 128
                                ],
                                rhs=xcs[kd][:, :],
                                start=(kd == 0),
                                stop=(kd == KD - 1),
                            )
                        if t < 2:
                            qa = actp.tile([64, 512], bf16, tag="qT", bufs=6,
                                           name="qa")
                            qb = actp.tile([64, 512], bf16, tag="qT", bufs=6,
                                           name="qb")
                            nc.vector.tensor_copy(qa[:, :], ps_p[0:64, :])
                            nc.vector.tensor_copy(qb[:, :], ps_p[64:128, :])
                            qcs += [qa, qb]
                        else:
                            kc = actp.tile([64, 512], bf16, tag="qT", bufs=6,
                                           name="kc")
                            vc = actp.tile([64, 512], bf16, tag="vT", bufs=4,
                                           name="vc")
                            nc.vector.tensor_copy(kc[:, :], ps_p[0:64, :])
                            nc.vector.tensor_copy(vc[:, :], ps_p[64:128, :])

                    # ---- RoPE for chunk c (DVE + GpSimd split) ----
                    rqs = []
                    for src in qcs + [kc]:
                        dst = actp.tile(
                            [64, 512], bf16,
                            tag="rqT" if src is not kc else "rkT",
                            bufs=10 if src is not kc else 8,
                            name="rq",
                        )
                        t1 = ropep.tile([32, 512], f32, tag="rt", name="t1")
                        t2 = ropep.tile([32, 512], f32, tag="rt", name="t2")
                        t3 = ropep.tile([32, 512], f32, tag="rt", name="t3")
                        t4 = ropep.tile([32, 512], f32, tag="rt", name="t4")
                        nc.vector.tensor_mul(
                            t1[:, :], src[0:32, :], ce_sb[0:32, cs]
                        )
                        nc.vector.tensor_mul(
                            t2[:, :], src[32:64, :], se_sb[32:64, cs]
                        )
                        gps.tensor_mul(t3[:, :], src[0:32, :], se_sb[0:32, cs])
                        gps.tensor_mul(t4[:, :], src[32:64, :], ce_sb[32:64, cs])
                        nc.vector.tensor_sub(dst[0:32, :], t1[:, :], t2[:, :])
                        nc.vector.tensor_add(dst[32:64, :], t3[:, :], t4[:, :])
                        rqs.append(dst)
                    rkTs.append(rqs[NH])

                    # ---- V transpose to token-major V_aug for chunk c ----
                    vsb = actp.tile([128, 4 * 68], bf16, tag="vsb", bufs=8,
                                    name="vsb")
                    nc.vector.memset(
                        vsb[:, :].rearrange("p (n j) -> p n j", n=4)[:, :, 64:65],
                        1.0,
                    )
                    for ts in range(4):
                        ps_t = psp.tile([128, 64], bf16, tag="ss", bufs=4,
                                        name="ps_t")
                        nc.tensor.transpose(
                            ps_t[:, :],
                            vc[:, ts * 128 : ts * 128 + 128],
                            ident[:, :],
                        )
                        nc.vector.tensor_copy(
                            vsb[:, ts * 68 : ts * 68 + 64], ps_t[:, :]
                        )
                    vsbs.append(vsb)

                    # ---- attention for query chunk j=c, all heads ----
                    # software-pipelined by one head: scores of head h are
                    # emitted before PV of head h-1, so ScalarE's exp of head
                    # h's probabilities overlaps PE's PV of head h-1
                    j = c
                    nts = 4 * j + 4

                    def emit_scores(h):
                        pts = []
                        for i in range(nts):
                            ci, si = divmod(i, 4)
                            cd = i - 4 * j
                            vs = max(cd, 0) * 128
                            ps_s = psp.tile([128, 512], f32, tag="ss", bufs=4,
                                            name="ps_s")
                            nc.tensor.matmul(
                                ps_s[:, vs:512],
                                lhsT=rkTs[ci][:, si * 128 : si * 128 + 128],
                                rhs=rqs[h][:, vs:512],
                                start=True,
                                stop=True,
                            )
                            pt = ptp.tile([128, 512], bf16, tag="pt", name="pt")
                            pts.append(pt)
                            if cd >= 1:
                                gps.memset(pt[:, 0:vs], 0.0)
                            nc.scalar.activation(
                                pt[:, vs:512], ps_s[:, vs:512], Exp,
                                scale=SCALE,
                            )
                            if cd >= 0:
                                # causal zeroing of the diagonal subtile:
                                # keep where tk(p) <= tq(f), else 0
                                nc.gpsimd.affine_select(
                                    out=pt[:, vs : vs + 128],
                                    in_=pt[:, vs : vs + 128],
                                    compare_op=mybir.AluOpType.is_ge,
                                    fill=0.0,
                                    base=0,
                                    pattern=[[1, 128]],
                                    channel_multiplier=-1,
                                )
                        return pts

                    def emit_pv(h, pts):
                        ps_pv = psp.tile([128, 512], f32, tag="pv", bufs=2,
                                         name="ps_pv")
                        for i in range(nts):
                            nc.tensor.matmul(
                                ps_pv[0:65, :],
                                lhsT=vsbs[i // 4][
                                    :, (i % 4) * 68 : (i % 4) * 68 + 65
                                ],
                                rhs=pts[i][:, :],
                                start=(i == 0),
                                stop=(i == nts - 1),
                            )
                        rd = smlp.tile([1, 512], bf16, tag="rd", name="rd")
                        with nc.allow_low_precision(
                            reason="softmax denom reciprocal in bf16 is "
                            "well within the rel-err budget"
                        ):
                            nc.vector.reciprocal(rd[:, :], ps_pv[64:65, :])
                        ps_rep = psp.tile([64, 512], f32, tag="ss", bufs=4,
                                          name="ps_rep")
                        nc.tensor.matmul(
                            ps_rep[:, :],
                            lhsT=ones[0:1, 0:64],
                            rhs=rd[:, :],
                            start=True,
                            stop=True,
                        )
                        ob = smlp.tile([64, 512], bf16, tag="ob", name="ob")
                        nc.vector.tensor_copy(ob[:, :], ps_pv[0:64, :])
                        oT = actp.tile([64, 512], bf16, tag="oT", bufs=6,
                                       name="oT")
                        nc.vector.tensor_mul(oT[:, :], ob[:, :], ps_rep[:, :])
                        # scatter the two 256-token halves to their slices
                        for half in range(2):
                            s_idx = 2 * j + half
                            nc.sync.dma_start(
                                out=ccin[
                                    s_idx * 256 + h * HD :
                                    s_idx * 256 + h * HD + HD,
                                    b * 256 : b * 256 + 256,
                                ],
                                in_=oT[:, half * 256 : half * 256 + 256],
                            )

                    prev = None
                    for h in range(NH):
                        pts_h = emit_scores(h)
                        if prev is not None:
                            emit_pv(prev[0], prev[1])
                        prev = (h, pts_h)
                    emit_pv(prev[0], prev[1])

            # ---- AllToAll: my heads everywhere -> all heads, my slice ----
            nc.gpsimd.collective_compute(
                "AllToAll",
                mybir.AluOpType.bypass,
                replica_groups=[list(range(8))],
                ins=[ccin[:, :]],
                outs=[ccout[:, :]],
            )
            oa_tiles = []
            for kd in range(KD):
                oa = oap.tile([128, 512], bf16, tag="oa", name="oa")
                nc.sync.dma_start(
                    out=oa[:, :], in_=ccout[kd * 128 : kd * 128 + 128, :]
                )
                oa_tiles.append(oa)

            # ---- output projection: full E, token shard, wo streamed ----
            for wc in range(4):
                wos = []
                for kd in range(KD):
                    wo = wop.tile([128, 512], bf16, tag="wo", name="wo")
                    nc.sync.dma_start(
                        out=wo[:, :],
                        in_=wo_d[
                            kd * 128 : kd * 128 + 128, wc * 512 : wc * 512 + 512
                        ],
                    )
                    wos.append(wo)
                for tt in range(4):
                    ps_o = psp.tile(
                        [128, 512], f32,
                        tag="ps" if (wc + tt) % 2 == 0 else "pv",
                        bufs=2, name="ps_o",
                    )
                    for kd in range(KD):
                        nc.tensor.matmul(
                            ps_o[:, :],
                            lhsT=oa_tiles[kd][:, tt * 128 : tt * 128 + 128],
                            rhs=wos[kd][:, :],
                            start=(kd == 0),
                            stop=(kd == KD - 1),
                        )
                    ot = ostp.tile([128, 512], f32, tag="ot", name="ot")
                    nc.any.tensor_copy(ot[:, :], ps_o[:, :])
                    nc.sync.dma_start(
                        out=out_d[
                            tt * 128 : tt * 128 + 128,
                            wc * 512 : wc * 512 + 512,
                        ],
                        in_=ot[:, :],
                    )
    nc.compile()
    return nc


_DEINT = np.concatenate([np.arange(0, HD, 2), np.arange(1, HD, 2)])


def _prep_in_maps(inputs):
    x = np.asarray(inputs["x"], dtype=np.float32)
    fc = np.asarray(inputs["freqs_cos"], dtype=np.float32)
    fs = np.asarray(inputs["freqs_sin"], dtype=np.float32)
    wq = np.asarray(inputs["wq"], dtype=np.float32)
    wk = np.asarray(inputs["wk"], dtype=np.float32)
    wv = np.asarray(inputs["wv"], dtype=np.float32)
    wo = np.asarray(inputs["wo"], dtype=np.float32)

    xT = np.ascontiguousarray(np.transpose(x, (0, 2, 1))).astype(BF16)
    ce = np.ascontiguousarray(
        np.concatenate([fc.T, fc.T], axis=0)
    ).astype(BF16)  # [64, S], table duplicated in both halves
    se = np.ascontiguousarray(
        np.concatenate([fs.T, fs.T], axis=0)
    ).astype(BF16)
    wof = np.ascontiguousarray(wo).astype(BF16)

    in_maps = []
    for m in range(8):
        cols = []
        for h in range(NH):
            qh = wq[:, (4 * m + h) * HD : (4 * m + h) * HD + HD]
            cols.append(qh[:, _DEINT])
        kh = wk[:, m * HD : m * HD + HD]
        cols.append(kh[:, _DEINT])
        cols.append(wv[:, m * HD : m * HD + HD])
        wqkv = np.ascontiguousarray(np.concatenate(cols, axis=1)).astype(BF16)
        in_maps.append(dict(xT=xT, wqkv=wqkv, wo_f=wof, ce=ce, se=se))
    return in_maps


LAST_EXEC_NS = None


def kernel(**inputs):
    global LAST_EXEC_NS
    from concourse import bass_utils

    in_maps = _prep_in_maps(inputs)
    nc = _build_bass()
    trace = bool(int(os.environ.get("KERNEL_TRACE", "0")))
    res = bass_utils.run_bass_kernel_spmd(
        nc, in_maps, core_ids=list(range(8)), trace=trace
    )
    if trace and res.exec_time_ns is not None:
        LAST_EXEC_NS = res.exec_time_ns
        print(f"HW exec time: {res.exec_time_ns} ns")
    out = np.empty((B, S, D), dtype=np.float32)
    for m, r in enumerate(res.results):
        o = r["out"]  # [512, D]: my 256 tokens of each batch
        for b in range(B):
            out[b, m * 256 : m * 256 + 256, :] = o[b * 256 : b * 256 + 256, :]
    return out


def time_device(reps=6, **inputs):
    """Wall-clock the sharded PJRT executable with device-resident inputs.

    NOTE: on the axon tunnel this is dominated by a ~45-75ms fixed RPC
    dispatch overhead and does not reflect kernel quality; the NTFF-traced
    exec time (KERNEL_TRACE=1) is the honest hardware number.
    """
    import jax
    from concourse import bass2jax
    import concourse.mybir as mybir
    import time as _time

    in_maps = _prep_in_maps(inputs)
    nc = _build_bass()
    bass2jax.install_neuronx_cc_hook()

    partition_name = (
        nc.partition_id_tensor.name if nc.partition_id_tensor else None
    )
    in_names, out_names, out_avals, zero_outs = [], [], [], []
    for alloc in nc.m.functions[0].allocations:
        if not isinstance(alloc, mybir.MemoryLocationSet):
            continue
        name = alloc.memorylocations[0].name
        if alloc.kind == "ExternalInput":
            if name != partition_name:
                in_names.append(name)
        elif alloc.kind == "ExternalOutput":
            out_names.append(name)
            shape = tuple(alloc.tensor_shape)
            dt = mybir.dt.np(alloc.dtype)
            out_avals.append(jax.core.ShapedArray(shape, dt))
            zero_outs.append(np.zeros(shape, dt))
    n_params = len(in_names)
    in_all = in_names + out_names
    if partition_name is not None:
        in_all = in_all + [partition_name]

    def _body(*args):
        operands = list(args)
        if partition_name is not None:
            operands.append(bass2jax.partition_id_tensor())
        outs = bass2jax._bass_exec_p.bind(
            *operands,
            out_avals=tuple(out_avals),
            in_names=tuple(in_all),
            out_names=tuple(out_names),
            lowering_input_output_aliases=(),
            sim_require_finite=True,
            sim_require_nnan=True,
            nc=nc,
        )
        return tuple(outs)

    devices = jax.devices()[:8]
    mesh = bass2jax.Mesh(np.asarray(devices), ("core",))
    spec = bass2jax.PartitionSpec("core")
    nin = n_params + len(out_names)
    f = jax.jit(
        bass2jax.shard_map(
            _body,
            mesh=mesh,
            in_specs=(spec,) * nin,
            out_specs=(spec,) * len(out_names),
            check_rep=False,
        )
    )
    concat_in = [
        np.concatenate([np.asarray(m[n]) for m in in_maps], axis=0)
        for n in in_names
    ]
    concat_zeros = [
        np.zeros((8 * z.shape[0], *z.shape[1:]), z.dtype) for z in zero_outs
    ]
    sharding = jax.sharding.NamedSharding(mesh, spec)
    dev_args = [jax.device_put(a, sharding) for a in concat_in + concat_zeros]
    r = f(*dev_args)
    jax.block_until_ready(r)
    best = None
    for _ in range(reps):
        t0 = _time.perf_counter()
        r = f(*dev_args)
        jax.block_until_ready(r)
        dt = _time.perf_counter() - t0
        best = dt if best is None else min(best, dt)
    return int(best * 1e9)
